# revision 27
# baseline (speedup 1.0000x reference)
"""CoAtNet relative attention kernel for Trainium2 (Bass/Tile), 8 NeuronCores.

Problem (per full input):
  x [16, 256, 32, 32] f32; Wq/Wk/Wv [256, 256]; Wo [256, 256]; bo [256];
  rel_bias [8, 3969]; rel_idx [1024, 1024] int32 (static pattern).
  out[b] = softmax(q k^T / sqrt(d) + bias) v  projected back, heads=8, d=32.

Sharding: data-parallel over batch - each of the 8 cores handles 2 batches
with identical programs (SPMD, no collectives).

Structure (v2 - ScalarE-bound pipeline, gap-free):
  * rel_idx[p, q] == (q - p) + 1056 exactly, so the bias is Toeplitz and
    exp(bias) is precomputed on host as a sheared tile G[h, i, j'] =
    exp(rel_bias[h, 1952 + i - j']) of shape [128, 1920] per head; applying
    it is a bf16 2x-mode multiply after exp(S): exp(S+B) = exp(S)*exp(B).
  * Everything is transposed so no transposes are needed: scores are built
    as S^T [keys, queries], P@V uses lhsT = V directly, and the final
    projection emits out^T [c, n] which is the output memory layout.
  * The softmax exp is the hard floor: 16.8M elements/core through ScalarE
    at 1 elem/lane/cycle = ~110us.  Everything else is arranged so ScalarE
    never waits: heads are processed in groups of 3/3/2 so an S^T strip is
    [128, 1536] f32 = 3 PSUM banks, which allows DOUBLE-BUFFERING the strip
    banks (6) next to the PV accumulators ot/den (2).  The QK^T matmuls for
    strip i+1 run while ScalarE exps strip i.
  * PV accumulates [V] rows (3 col-tiled M=32 matmuls) and the softmax
    denominators (ones lhsT) into one ot and one den bank across the 8
    key-tiles; the normalization is bounce-free: reciprocal_approx_accurate
    reads den straight from PSUM (den rows are replicated 32x per head so
    they line up 1:1 with ot rows) and one tensor_mul writes the normalized
    out^T slice.
  * b1's projections are dripped into b0's 2-head phases (where the strip
    tile has a spare PSUM bank at [:, 1024:1536]); b0's output projection is
    dripped into b1's 2-head phases the same way.  The output projection
    adds the bias via a K=1 ones-row matmul so ScalarE is not involved.
"""

import numpy as np
from contextlib import ExitStack

import concourse.bass as bass
import concourse.bacc as bacc
import concourse.mybir as mybir
import concourse.tile as tile
from concourse import bass_utils
from concourse._compat import with_exitstack

HEADS = 8
D = 32  # head dim
C = 256  # channels = heads * D
N = 1024  # tokens = 32 * 32
B_LOC = 2  # batches per core
N_CORES = 8
SCALE = D ** -0.5
GW = 1920  # sheared bias tile width
G0 = 1952  # G[h, i, j'] = rel_bias[h, G0 + i - j']

GRP = [(0, 3), (3, 3), (6, 2)]  # (first head, n heads) per group

F32 = mybir.dt.float32
BF16 = mybir.dt.bfloat16
AF = mybir.ActivationFunctionType


@with_exitstack
def _emit(ctx: ExitStack, tc: tile.TileContext, io: dict):
    nc = tc.nc
    x, wqT, wkT, wvT, woT, bo, eb, out = (
        io[k] for k in ("x", "wqT", "wkT", "wvT", "woT", "bo", "eb", "out")
    )

    persist = ctx.enter_context(tc.tile_pool(name="persist", bufs=1))
    se_pool = ctx.enter_context(tc.tile_pool(name="se", bufs=9))
    nrm_pool = ctx.enter_context(tc.tile_pool(name="nrm", bufs=2))
    outp = ctx.enter_context(tc.tile_pool(name="outp", bufs=3))
    # PSUM: st [128,1536] x2 (6 banks) + ot (1) + den (1) = 8.
    ps_st = ctx.enter_context(tc.tile_pool(name="ps_st", bufs=2, space="PSUM"))
    ps_ot = ctx.enter_context(tc.tile_pool(name="ps_ot", bufs=1, space="PSUM"))
    ps_den = ctx.enter_context(tc.tile_pool(name="ps_den", bufs=1, space="PSUM"))

    # ---------- input DMAs, ordered by first use, spread over queues ----------
    wq_sb, wk_sb, wv_sb, wo_sb = [], [], [], []
    x_sb = [[persist.tile([128, N], BF16, tag=f"x{b}_{cc}", name=f"x{b}_{cc}") for cc in range(2)] for b in range(B_LOC)]
    nc.scalar.dma_start(out=x_sb[0][0][:], in_=x[0, 0:128, :])
    for lst, src, nm in ((wq_sb, wqT, "wq"), (wk_sb, wkT, "wk")):
        for cc in range(2):
            t = persist.tile([128, C], BF16, tag=f"{nm}{cc}", name=f"{nm}{cc}")
            nc.sync.dma_start(out=t[:], in_=src[128 * cc : 128 * (cc + 1), :])
            lst.append(t)
    nc.sync.dma_start(out=x_sb[0][1][:], in_=x[0, 128:256, :])
    for cc in range(2):
        t = persist.tile([128, C], BF16, tag=f"wv{cc}", name=f"wv{cc}")
        nc.scalar.dma_start(out=t[:], in_=wvT[128 * cc : 128 * (cc + 1), :])
        wv_sb.append(t)
    nc.gpsimd.dma_start(out=x_sb[1][0][:], in_=x[1, 0:128, :])
    nc.gpsimd.dma_start(out=x_sb[1][1][:], in_=x[1, 128:256, :])
    # exp-of-bias sheared tiles, one DMA per head, earliest heads first
    eb_sb = persist.tile([128, HEADS * GW], BF16, tag="eb", name="eb_sb")
    eb_eng = [nc.sync, nc.scalar, nc.sync, nc.scalar, nc.sync, nc.scalar, nc.gpsimd, nc.gpsimd]
    for h in range(HEADS):
        eb_eng[h].dma_start(out=eb_sb[:, GW * h : GW * (h + 1)], in_=eb[h])
    for cc in range(2):
        t = persist.tile([128, C], BF16, tag=f"wo{cc}", name=f"wo{cc}")
        nc.gpsimd.dma_start(out=t[:], in_=woT[128 * cc : 128 * (cc + 1), :])
        wo_sb.append(t)
    bo_sb = persist.tile([1, C], BF16, tag="bo", name="bo_sb")
    nc.gpsimd.dma_start(out=bo_sb[:], in_=bo[:])

    ones32_sb = persist.tile([128, 32], BF16, tag="ones32", name="ones32")
    nc.vector.memset(ones32_sb[:], 1.0)
    ones_row = persist.tile([1, 512], BF16, tag="ones_row", name="ones_row")
    nc.vector.memset(ones_row[:], 1.0)

    # ---------- persistent intermediates ----------
    qT_sb = [[persist.tile([128, N], BF16, tag=f"qT{b}_{oc}", name=f"qT{b}_{oc}") for oc in range(2)] for b in range(B_LOC)]
    kT_sb = [[persist.tile([128, N], BF16, tag=f"kT{b}_{oc}", name=f"kT{b}_{oc}") for oc in range(2)] for b in range(B_LOC)]
    v_sb = [[persist.tile([128, 33 * HEADS], BF16, tag=f"v{b}_{nt}", name=f"v{b}_{nt}") for nt in range(8)] for b in range(B_LOC)]
    otn_sb = [[persist.tile([128, N], BF16, tag=f"otn{b}_{ch}", name=f"otn{b}_{ch}") for ch in range(2)] for b in range(B_LOC)]

    # ---------- projection / output-projection groups ----------
    def emit_qk_group(b, oc, nc2, w_sb, dst, po):
        # po: [128, 512] PSUM slice
        for cc in range(2):
            nc.tensor.matmul(
                po[:],
                lhsT=w_sb[cc][:, 128 * oc : 128 * (oc + 1)],
                rhs=x_sb[b][cc][:, 512 * nc2 : 512 * (nc2 + 1)],
                start=(cc == 0),
                stop=(cc == 1),
            )
        nc.vector.tensor_copy(
            out=dst[b][oc][:, 512 * nc2 : 512 * (nc2 + 1)], in_=po[:]
        )

    def emit_v_group(b, nt, po):
        for cc in range(2):
            nc.tensor.matmul(
                po[:, 0:C],
                lhsT=x_sb[b][cc][:, 128 * nt : 128 * (nt + 1)],
                rhs=wv_sb[cc][:],
                start=(cc == 0),
                stop=(cc == 1),
            )
        v33 = v_sb[b][nt][:].rearrange("p (h w) -> p h w", w=33)
        nc.vector.tensor_copy(
            out=v33[:, :, 0:32], in_=po[:, 0:C].rearrange("p (h w) -> p h w", w=32)
        )
        nc.vector.memset(v33[:, :, 32:33], 1.0)

    def stage_c_group(b, ct, q2, po, eng=None):
        # out^T[c-chunk, q-chunk] = Wo^T-chunk @ otn + bo (K=1 ones-row matmul)
        for ch in range(2):
            nc.tensor.matmul(
                po[:],
                lhsT=wo_sb[ch][:, 128 * ct : 128 * (ct + 1)],
                rhs=otn_sb[b][ch][:, 512 * q2 : 512 * (q2 + 1)],
                start=(ch == 0),
                stop=False,
            )
        nc.tensor.matmul(
            po[:],
            lhsT=bo_sb[0:1, 128 * ct : 128 * (ct + 1)],
            rhs=ones_row[0:1, :],
            start=False,
            stop=True,
        )
        ob = outp.tile([128, 512], F32, tag="ob", name="ob_t")
        nc.vector.tensor_copy(out=ob[:], in_=po[:])
        (eng or nc.sync).dma_start(
            out=out[b, 128 * ct : 128 * (ct + 1), 512 * q2 : 512 * (q2 + 1)],
            in_=ob[:],
        )

    # ---------- batch-0 projections upfront (also a PE warm-up burst) ----------
    def st_slices():
        # rotate 8 [128,512] psum slots (2 st tiles x3 + ot + den) so the
        # trailing DVE casts never gate the next projection matmuls
        while True:
            stt = ps_st.tile([128, 1536], F32, tag="st", name="st_ps")
            for s in range(3):
                yield stt[:, 512 * s : 512 * (s + 1)]
            yield ps_ot.tile([128, 512], F32, tag="ot", name="ot_ps")[:]
            stt = ps_st.tile([128, 1536], F32, tag="st", name="st_ps")
            for s in range(3):
                yield stt[:, 512 * s : 512 * (s + 1)]
            yield ps_den.tile([128, 512], F32, tag="den", name="den_ps")[:]

    # Minimal prefix before attention can start: qT/kT chunk 0 (heads 0-3),
    # kT chunk 1 keys 0-511, and v key-tiles 0-1.  The rest of batch 0's
    # projections drip into the first two phases' ot/den-bank windows.
    slc = st_slices()
    next(slc)  # skip stA slice 0 so ST(phase0, kt0)'s tile has no WAR on us
    next(slc)
    next(slc)
    for nc2 in range(2):
        for w_sb, dst in ((wq_sb, qT_sb), (wk_sb, kT_sb)):
            emit_qk_group(0, 0, nc2, w_sb, dst, next(slc))
    emit_qk_group(0, 1, 0, wk_sb, kT_sb, next(slc))
    emit_v_group(0, 0, next(slc))

    # ---------- drip schedules (work that borrows a spare PSUM bank) ----------
    # phases 0/1: the rest of b0's projections, into the ot/den banks while
    # those are between the previous phase's norm and this phase's first PV
    # p0 cells kt0-6 and p1 cells kt3-6 have the ot/den banks free (the
    # previous phase's PVs/norm occupy cells 0-2; this phase's first PV
    # lands at cell 6 with start=True, overwriting any drip leftovers).
    p01_drips = {
        (0, kt): (lambda po, nt=1 + kt: emit_v_group(0, nt, po)) for kt in range(7)
    }
    p01_drips[(1, 3)] = lambda po: emit_qk_group(0, 1, 0, wq_sb, qT_sb, po)
    p01_drips[(1, 4)] = lambda po: emit_qk_group(0, 1, 1, wk_sb, kT_sb, po)
    p01_drips[(1, 5)] = lambda po: emit_qk_group(0, 1, 1, wq_sb, qT_sb, po)
    # b0's 2-head phases: b1 projections (oc0 first; G1 needs oc1 before b1-G1)
    b0_drips = []
    for oc in range(2):
        for nc2 in range(2):
            for w_sb, dst in ((wq_sb, qT_sb), (wk_sb, kT_sb)):
                b0_drips.append(
                    lambda po, oc=oc, nc2=nc2, w_sb=w_sb, dst=dst: emit_qk_group(
                        1, oc, nc2, w_sb, dst, po
                    )
                )
    for nt in range(8):
        b0_drips.append(lambda po, nt=nt: emit_v_group(1, nt, po))
    # b1's 2-head phases: b0 output projection, then b1's first-half outputs
    b1_drips = {
        (0, 0): lambda po: stage_c_group(0, 0, 0, po),
        (0, 2): lambda po: stage_c_group(0, 0, 1, po, eng=nc.scalar),
        (0, 4): lambda po: stage_c_group(0, 1, 0, po, eng=nc.gpsimd),
        (0, 6): lambda po: stage_c_group(0, 1, 1, po),
        (1, 3): lambda po: stage_c_group(1, 0, 0, po, eng=nc.gpsimd),
        (1, 5): lambda po: stage_c_group(1, 1, 0, po, eng=nc.scalar),
    }

    # ---------- attention ----------
    eb3 = eb_sb[:].rearrange("p (h w) -> p h w", w=GW)
    pending_pv = None

    def emit_pv(args):
        b_, hl_, ng_, kt_, se_, ot_, den_, first, last = args
        for j in range(ng_):
            nc.tensor.matmul(
                ot_[32 * j : 32 * (j + 1), :],
                lhsT=v_sb[b_][kt_][:, 33 * (hl_ + j) : 33 * (hl_ + j) + 32],
                rhs=se_[:, 512 * j : 512 * (j + 1)],
                start=first,
                stop=last,
                tile_position=(0, 32 * j),
                skip_group_check=True,
            )
        for j in range(ng_):
            nc.tensor.matmul(
                den_[32 * j : 32 * (j + 1), :],
                lhsT=ones32_sb[:],
                rhs=se_[:, 512 * j : 512 * (j + 1)],
                start=first,
                stop=last,
                tile_position=(0, 32 * j),
                skip_group_check=True,
            )

    # otn destination mapping: group g rows -> (chunk, chunk-rows, ot-rows)
    OTN_MAP = [
        [(0, 0, 96, 0)],          # G0 -> chunk0 rows 0..96
        [(0, 96, 128, 0), (1, 0, 32, 32), (1, 32, 64, 64)],  # G1 -> chunk0 96..128 + chunk1 0..64
        [(1, 64, 128, 0)],        # G2 -> chunk1 64..128
    ]

    def make_norm(b, g, qi, ot_, den_):
        # Split in two drip-able parts so the DVE chain never sits ahead of
        # the next phase's bias-muls in the queue.
        hl, ng = GRP[g]
        rows = 32 * ng
        state = {}

        def part1():
            rden = nrm_pool.tile([128, 512], F32, tag="rden", name="rden_t")
            rscr = nrm_pool.tile([128, 512], F32, tag="rscr", name="rscr_t")
            nc.vector.reciprocal_approx_accurate(
                out=rden[0:rows, :], in_=den_[0:rows, :], scratch=rscr[0:rows, :]
            )
            state["rden"] = rden

        def part2():
            rden = state["rden"]
            for ch, r0, r1, s0 in OTN_MAP[g]:
                nc.vector.tensor_mul(
                    out=otn_sb[b][ch][r0:r1, 512 * qi : 512 * (qi + 1)],
                    in0=ot_[s0 : s0 + (r1 - r0), :],
                    in1=rden[s0 : s0 + (r1 - r0), :],
                )

        return [part1, part2]

    # Deferred-work queue drained <=2 items per strip-iteration, so the
    # trailing PV packs and norm ops of a phase interleave with the NEXT
    # phase's QK^T matmuls instead of piling up ahead of them at the
    # boundary.  Items: ("pv", args, eligible_iter) / ("fn", closure, iter).
    # PV of strip kt is eligible at kt+2 (its bias-mul is then guaranteed
    # complete, so it never head-of-line blocks the PE queue); the kt==0 PV
    # additionally sits behind the previous phase's norm (+3).
    evq = []
    it = 0

    def drain(limit=2):
        # limit counts PV packs only; norm closures (DVE work) drain freely
        n = 0
        while evq and n < limit and evq[0][2] <= it:
            kind, payload, _ = evq.pop(0)
            if kind == "pv":
                emit_pv(payload)
                n += 1
            else:
                payload()

    phases = [(b, g, qi) for b in range(B_LOC) for g in range(3) for qi in range(2)]
    for pi, (b, g, qi) in enumerate(phases):
        hl, ng = GRP[g]
        w = 512 * ng
        last_phase = pi == len(phases) - 1
        ot_ps = ps_ot.tile([128, 512], F32, tag="ot", name="ot_ps")
        den_ps = ps_den.tile([128, 512], F32, tag="den", name="den_ps")
        for kt in range(8):
            stt = ps_st.tile([128, 1536], F32, tag="st", name="st_ps")
            for j in range(ng):
                h = hl + j
                ch, band = h // 4, 32 * (h % 4)
                nc.tensor.matmul(
                    stt[:, 512 * j : 512 * (j + 1)],
                    lhsT=kT_sb[b][ch][band : band + 32, 128 * kt : 128 * (kt + 1)],
                    rhs=qT_sb[b][ch][band : band + 32, 512 * qi : 512 * (qi + 1)],
                    start=True,
                    stop=True,
                    tile_position=(band, 0),
                )
            if g == 2:
                drip = None
                if b == 0 and b0_drips:
                    drip = b0_drips.pop(0)
                elif b == 1:
                    drip = b1_drips.pop((qi, kt), None)
                if drip is not None:
                    drip(stt[:, 1024:1536])
            elif pi < 2:
                drip = p01_drips.pop((pi, kt), None)
                if drip is not None:
                    # alternate the two accumulator banks; PVs of this phase
                    # start only at cell 6, after which the banks are theirs
                    drip(ot_ps[:] if kt % 2 == 0 else den_ps[:])
            se = se_pool.tile([128, 1536], BF16, tag="se", name="se_t")
            nc.scalar.activation(out=se[:, 0:w], in_=stt[:, 0:w], func=AF.Exp)
            off = 896 - 128 * kt + 512 * qi
            se3 = se[:, 0:w].rearrange("p (j q) -> p j q", q=512)
            nc.vector.tensor_mul(
                out=se3, in0=se3, in1=eb3[:, hl : hl + ng, off : off + 512]
            )
            drain(3 if pi >= len(phases) - 2 else 1)
            if pi < 2:
                elig = it - kt + max(kt + 2, 6)
            elif last_phase and kt >= 5:
                elig = it + 1
            else:
                elig = it + (3 if kt == 0 else 2)
            evq.append(("pv", (b, hl, ng, kt, se, ot_ps, den_ps, kt == 0, kt == 7),
                        elig))
            it += 1
        if last_phase:
            while evq:
                kind, payload, _ = evq.pop(0)
                emit_pv(payload) if kind == "pv" else payload()
            for p in make_norm(b, g, qi, ot_ps, den_ps):
                p()
        else:
            for p in make_norm(b, g, qi, ot_ps, den_ps):
                evq.append(("fn", p, it))

    # ---------- tail: b1's second-half outputs ----------
    stage_c_group(1, 0, 1, ps_ot.tile([128, 512], F32, tag="ot", name="ot_ps"))
    stage_c_group(1, 1, 1, ps_den.tile([128, 512], F32, tag="den", name="den_ps"),
                  eng=nc.scalar)


def build():
    nc = bacc.Bacc("TRN2", target_bir_lowering=False, debug=False, num_devices=N_CORES)
    io = {
        "x": nc.dram_tensor("x", [B_LOC, C, N], BF16, kind="ExternalInput").ap(),
        "wqT": nc.dram_tensor("wqT", [C, C], BF16, kind="ExternalInput").ap(),
        "wkT": nc.dram_tensor("wkT", [C, C], BF16, kind="ExternalInput").ap(),
        "wvT": nc.dram_tensor("wvT", [C, C], BF16, kind="ExternalInput").ap(),
        "woT": nc.dram_tensor("woT", [C, C], BF16, kind="ExternalInput").ap(),
        "bo": nc.dram_tensor("bo", [1, C], BF16, kind="ExternalInput").ap(),
        "eb": nc.dram_tensor("eb", [HEADS, 128, GW], BF16, kind="ExternalInput").ap(),
        "out": nc.dram_tensor("out", [B_LOC, C, N], F32, kind="ExternalOutput").ap(),
    }
    with tile.TileContext(nc) as tc:
        _emit(tc, io)
    nc.compile()
    return nc


_CACHE: dict = {}


def _get_nc():
    if "nc" not in _CACHE:
        _CACHE["nc"] = build()
    return _CACHE["nc"]


def make_in_maps(x, Wq, Wk, Wv, Wo, bo, rel_bias, rel_idx=None):
    """Host-side sharding/layout prep. Returns per-core input maps."""
    import ml_dtypes

    bf16 = ml_dtypes.bfloat16
    x = np.asarray(x, np.float32)
    b, c, H, W = x.shape
    assert (b, c, H * W) == (B_LOC * N_CORES, C, N)
    xr = np.ascontiguousarray(x.reshape(b, c, N).astype(bf16))
    wqT = np.ascontiguousarray(np.asarray(Wq, np.float32).T.astype(bf16))
    wkT = np.ascontiguousarray((np.asarray(Wk, np.float32) * SCALE).T.astype(bf16))
    wvT = np.ascontiguousarray(np.asarray(Wv, np.float32).T.astype(bf16))
    woT = np.ascontiguousarray(np.asarray(Wo, np.float32).T.astype(bf16))
    bo2 = np.ascontiguousarray(np.asarray(bo, np.float32).reshape(1, C).astype(bf16))
    rb = np.asarray(rel_bias, np.float32)
    idx = G0 + np.arange(128)[:, None] - np.arange(GW)[None, :]
    ebmat = np.ascontiguousarray(np.exp(rb[:, idx]).astype(bf16))  # [8, 128, GW]
    shared = dict(wqT=wqT, wkT=wkT, wvT=wvT, woT=woT, bo=bo2, eb=ebmat)
    return [
        dict(x=np.ascontiguousarray(xr[B_LOC * i : B_LOC * (i + 1)]), **shared)
        for i in range(N_CORES)
    ]


def _install_ntff_hook_shim():
    """bass_utils fetches the axon NTFF hook via antenv.axon_hooks, which this
    image's antenv lacks; synthesize it from trn_agent_boot's ctypes hook."""
    import sys
    import types

    try:
        from antenv.axon_hooks import get_axon_ntff_profile_hook  # noqa: F401

        return
    except ImportError:
        pass
    hook = None
    try:
        from trn_agent_boot.trn_boot import _ntff_profile_via_ctypes

        hook = _ntff_profile_via_ctypes("/opt/axon/libaxon_pjrt.so")
    except Exception:
        pass
    mod = types.ModuleType("antenv.axon_hooks")
    state = {"hook": hook}
    mod.get_axon_ntff_profile_hook = lambda: state["hook"]
    mod.set_axon_ntff_profile_hook = lambda h: state.__setitem__("hook", h)
    sys.modules["antenv.axon_hooks"] = mod


def run(inputs: dict, trace: bool = False):
    """Run on the 8 cores; returns (full_output, BassKernelResults)."""
    if trace:
        _install_ntff_hook_shim()
    in_maps = make_in_maps(**inputs)
    nc = _get_nc()
    res = bass_utils.run_bass_kernel_spmd(
        nc, in_maps, core_ids=list(range(N_CORES)), trace=trace
    )
    outs = np.stack([res.results[i]["out"] for i in range(N_CORES)])
    out = outs.reshape(B_LOC * N_CORES, C, 32, 32)
    return out, res


def kernel(**inputs) -> np.ndarray:
    out, _ = run(inputs)
    return out


# revision 28
# speedup vs baseline: 1.0101x; 1.0101x over previous
"""CoAtNet relative attention kernel for Trainium2 (Bass/Tile), 8 NeuronCores.

Problem (per full input):
  x [16, 256, 32, 32] f32; Wq/Wk/Wv [256, 256]; Wo [256, 256]; bo [256];
  rel_bias [8, 3969]; rel_idx [1024, 1024] int32 (static pattern).
  out[b] = softmax(q k^T / sqrt(d) + bias) v  projected back, heads=8, d=32.

Sharding: data-parallel over batch - each of the 8 cores handles 2 batches
with identical programs (SPMD, no collectives).

Structure (v2 - ScalarE-bound pipeline, gap-free):
  * rel_idx[p, q] == (q - p) + 1056 exactly, so the bias is Toeplitz and
    exp(bias) is precomputed on host as a sheared tile G[h, i, j'] =
    exp(rel_bias[h, 1952 + i - j']) of shape [128, 1920] per head; applying
    it is a bf16 2x-mode multiply after exp(S): exp(S+B) = exp(S)*exp(B).
  * Everything is transposed so no transposes are needed: scores are built
    as S^T [keys, queries], P@V uses lhsT = V directly, and the final
    projection emits out^T [c, n] which is the output memory layout.
  * The softmax exp is the hard floor: 16.8M elements/core through ScalarE
    at 1 elem/lane/cycle = ~110us.  Everything else is arranged so ScalarE
    never waits: heads are processed in groups of 3/3/2 so an S^T strip is
    [128, 1536] f32 = 3 PSUM banks, which allows DOUBLE-BUFFERING the strip
    banks (6) next to the PV accumulators ot/den (2).  The QK^T matmuls for
    strip i+1 run while ScalarE exps strip i.
  * PV accumulates [V] rows (3 col-tiled M=32 matmuls) and the softmax
    denominators (ones lhsT) into one ot and one den bank across the 8
    key-tiles; the normalization is bounce-free: reciprocal_approx_accurate
    reads den straight from PSUM (den rows are replicated 32x per head so
    they line up 1:1 with ot rows) and one tensor_mul writes the normalized
    out^T slice.
  * b1's projections are dripped into b0's 2-head phases (where the strip
    tile has a spare PSUM bank at [:, 1024:1536]); b0's output projection is
    dripped into b1's 2-head phases the same way.  The output projection
    adds the bias via a K=1 ones-row matmul so ScalarE is not involved.
"""

import numpy as np
from contextlib import ExitStack

import concourse.bass as bass
import concourse.bacc as bacc
import concourse.mybir as mybir
import concourse.tile as tile
from concourse import bass_utils
from concourse._compat import with_exitstack

HEADS = 8
D = 32  # head dim
C = 256  # channels = heads * D
N = 1024  # tokens = 32 * 32
B_LOC = 2  # batches per core
N_CORES = 8
SCALE = D ** -0.5
GW = 1920  # sheared bias tile width
G0 = 1952  # G[h, i, j'] = rel_bias[h, G0 + i - j']

GRP = [(0, 3), (3, 3), (6, 2)]  # (first head, n heads) per group

F32 = mybir.dt.float32
BF16 = mybir.dt.bfloat16
AF = mybir.ActivationFunctionType


@with_exitstack
def _emit(ctx: ExitStack, tc: tile.TileContext, io: dict):
    nc = tc.nc
    x, wqT, wkT, wvT, woT, bo, eb, out = (
        io[k] for k in ("x", "wqT", "wkT", "wvT", "woT", "bo", "eb", "out")
    )

    persist = ctx.enter_context(tc.tile_pool(name="persist", bufs=1))
    se_pool = ctx.enter_context(tc.tile_pool(name="se", bufs=9))
    nrm_pool = ctx.enter_context(tc.tile_pool(name="nrm", bufs=2))
    outp = ctx.enter_context(tc.tile_pool(name="outp", bufs=3))
    # PSUM: st [128,1536] x2 (6 banks) + ot (1) + den (1) = 8.
    ps_st = ctx.enter_context(tc.tile_pool(name="ps_st", bufs=2, space="PSUM"))
    ps_ot = ctx.enter_context(tc.tile_pool(name="ps_ot", bufs=1, space="PSUM"))
    ps_den = ctx.enter_context(tc.tile_pool(name="ps_den", bufs=1, space="PSUM"))

    # ---------- input DMAs, ordered by first use, spread over queues ----------
    wq_sb, wk_sb, wv_sb, wo_sb = [], [], [], []
    x_sb = [[persist.tile([128, N], BF16, tag=f"x{b}_{cc}", name=f"x{b}_{cc}") for cc in range(2)] for b in range(B_LOC)]
    nc.scalar.dma_start(out=x_sb[0][0][:], in_=x[0, 0:128, :])
    for lst, src, nm in ((wq_sb, wqT, "wq"), (wk_sb, wkT, "wk")):
        for cc in range(2):
            t = persist.tile([128, C], BF16, tag=f"{nm}{cc}", name=f"{nm}{cc}")
            nc.sync.dma_start(out=t[:], in_=src[128 * cc : 128 * (cc + 1), :])
            lst.append(t)
    nc.sync.dma_start(out=x_sb[0][1][:], in_=x[0, 128:256, :])
    for cc in range(2):
        t = persist.tile([128, C], BF16, tag=f"wv{cc}", name=f"wv{cc}")
        nc.scalar.dma_start(out=t[:], in_=wvT[128 * cc : 128 * (cc + 1), :])
        wv_sb.append(t)
    nc.gpsimd.dma_start(out=x_sb[1][0][:], in_=x[1, 0:128, :])
    nc.gpsimd.dma_start(out=x_sb[1][1][:], in_=x[1, 128:256, :])
    # exp-of-bias sheared tiles, one DMA per head, earliest heads first
    eb_sb = persist.tile([128, HEADS * GW], BF16, tag="eb", name="eb_sb")
    eb_eng = [nc.sync, nc.scalar, nc.sync, nc.scalar, nc.sync, nc.scalar, nc.gpsimd, nc.gpsimd]
    for h in range(HEADS):
        eb_eng[h].dma_start(out=eb_sb[:, GW * h : GW * (h + 1)], in_=eb[h])
    for cc in range(2):
        t = persist.tile([128, C], BF16, tag=f"wo{cc}", name=f"wo{cc}")
        nc.gpsimd.dma_start(out=t[:], in_=woT[128 * cc : 128 * (cc + 1), :])
        wo_sb.append(t)
    bo_sb = persist.tile([1, C], BF16, tag="bo", name="bo_sb")
    nc.gpsimd.dma_start(out=bo_sb[:], in_=bo[:])

    ones32_sb = persist.tile([128, 32], BF16, tag="ones32", name="ones32")
    nc.vector.memset(ones32_sb[:], 1.0)
    ones_row = persist.tile([1, 512], BF16, tag="ones_row", name="ones_row")
    nc.vector.memset(ones_row[:], 1.0)

    # ---------- persistent intermediates ----------
    qT_sb = [[persist.tile([128, N], BF16, tag=f"qT{b}_{oc}", name=f"qT{b}_{oc}") for oc in range(2)] for b in range(B_LOC)]
    kT_sb = [[persist.tile([128, N], BF16, tag=f"kT{b}_{oc}", name=f"kT{b}_{oc}") for oc in range(2)] for b in range(B_LOC)]
    v_sb = [[persist.tile([128, 33 * HEADS], BF16, tag=f"v{b}_{nt}", name=f"v{b}_{nt}") for nt in range(8)] for b in range(B_LOC)]
    otn_sb = [[persist.tile([128, N], BF16, tag=f"otn{b}_{ch}", name=f"otn{b}_{ch}") for ch in range(2)] for b in range(B_LOC)]

    # ---------- projection / output-projection groups ----------
    def emit_qk_group(b, oc, nc2, w_sb, dst, po):
        # po: [128, 512] PSUM slice
        for cc in range(2):
            nc.tensor.matmul(
                po[:],
                lhsT=w_sb[cc][:, 128 * oc : 128 * (oc + 1)],
                rhs=x_sb[b][cc][:, 512 * nc2 : 512 * (nc2 + 1)],
                start=(cc == 0),
                stop=(cc == 1),
            )
        nc.vector.tensor_copy(
            out=dst[b][oc][:, 512 * nc2 : 512 * (nc2 + 1)], in_=po[:]
        )

    def emit_v_group(b, nt, po):
        for cc in range(2):
            nc.tensor.matmul(
                po[:, 0:C],
                lhsT=x_sb[b][cc][:, 128 * nt : 128 * (nt + 1)],
                rhs=wv_sb[cc][:],
                start=(cc == 0),
                stop=(cc == 1),
            )
        v33 = v_sb[b][nt][:].rearrange("p (h w) -> p h w", w=33)
        nc.vector.tensor_copy(
            out=v33[:, :, 0:32], in_=po[:, 0:C].rearrange("p (h w) -> p h w", w=32)
        )
        nc.vector.memset(v33[:, :, 32:33], 1.0)

    def stage_c_group(b, ct, q2, po, eng=None):
        # out^T[c-chunk, q-chunk] = Wo^T-chunk @ otn + bo (K=1 ones-row matmul)
        for ch in range(2):
            nc.tensor.matmul(
                po[:],
                lhsT=wo_sb[ch][:, 128 * ct : 128 * (ct + 1)],
                rhs=otn_sb[b][ch][:, 512 * q2 : 512 * (q2 + 1)],
                start=(ch == 0),
                stop=False,
            )
        nc.tensor.matmul(
            po[:],
            lhsT=bo_sb[0:1, 128 * ct : 128 * (ct + 1)],
            rhs=ones_row[0:1, :],
            start=False,
            stop=True,
        )
        ob = outp.tile([128, 512], F32, tag="ob", name="ob_t")
        nc.vector.tensor_copy(out=ob[:], in_=po[:])
        (eng or nc.sync).dma_start(
            out=out[b, 128 * ct : 128 * (ct + 1), 512 * q2 : 512 * (q2 + 1)],
            in_=ob[:],
        )

    # ---------- batch-0 projections upfront (also a PE warm-up burst) ----------
    def st_slices():
        # rotate 8 [128,512] psum slots (2 st tiles x3 + ot + den) so the
        # trailing DVE casts never gate the next projection matmuls
        while True:
            stt = ps_st.tile([128, 1536], F32, tag="st", name="st_ps")
            for s in range(3):
                yield stt[:, 512 * s : 512 * (s + 1)]
            yield ps_ot.tile([128, 512], F32, tag="ot", name="ot_ps")[:]
            stt = ps_st.tile([128, 1536], F32, tag="st", name="st_ps")
            for s in range(3):
                yield stt[:, 512 * s : 512 * (s + 1)]
            yield ps_den.tile([128, 512], F32, tag="den", name="den_ps")[:]

    # Minimal prefix before attention can start: qT/kT chunk 0 (heads 0-3),
    # kT chunk 1 keys 0-511, and v key-tiles 0-1.  The rest of batch 0's
    # projections drip into the first two phases' ot/den-bank windows.
    slc = st_slices()
    next(slc)  # skip stA slice 0 so ST(phase0, kt0)'s tile has no WAR on us
    next(slc)
    next(slc)
    for nc2 in range(2):
        for w_sb, dst in ((wq_sb, qT_sb), (wk_sb, kT_sb)):
            emit_qk_group(0, 0, nc2, w_sb, dst, next(slc))
    emit_qk_group(0, 1, 0, wk_sb, kT_sb, next(slc))
    emit_v_group(0, 0, next(slc))

    # ---------- drip schedules (work that borrows a spare PSUM bank) ----------
    # phases 0/1: the rest of b0's projections, into the ot/den banks while
    # those are between the previous phase's norm and this phase's first PV
    # p0 cells kt0-6 and p1 cells kt3-6 have the ot/den banks free (the
    # previous phase's PVs/norm occupy cells 0-2; this phase's first PV
    # lands at cell 6 with start=True, overwriting any drip leftovers).
    p01_drips = {
        (0, kt): (lambda po, nt=1 + kt: emit_v_group(0, nt, po)) for kt in range(7)
    }
    p01_drips[(1, 3)] = lambda po: emit_qk_group(0, 1, 0, wq_sb, qT_sb, po)
    p01_drips[(1, 4)] = lambda po: emit_qk_group(0, 1, 1, wk_sb, kT_sb, po)
    p01_drips[(1, 5)] = lambda po: emit_qk_group(0, 1, 1, wq_sb, qT_sb, po)
    # b0's 2-head phases: b1 projections (oc0 first; G1 needs oc1 before b1-G1)
    b0_drips = []
    for oc in range(2):
        for nc2 in range(2):
            for w_sb, dst in ((wq_sb, qT_sb), (wk_sb, kT_sb)):
                b0_drips.append(
                    lambda po, oc=oc, nc2=nc2, w_sb=w_sb, dst=dst: emit_qk_group(
                        1, oc, nc2, w_sb, dst, po
                    )
                )
    for nt in range(8):
        b0_drips.append(lambda po, nt=nt: emit_v_group(1, nt, po))
    # b1's 2-head phases: b0 output projection, then b1's first-half outputs
    b1_drips = {
        (0, 0): lambda po: stage_c_group(0, 0, 0, po),
        (0, 2): lambda po: stage_c_group(0, 0, 1, po, eng=nc.scalar),
        (0, 4): lambda po: stage_c_group(0, 1, 0, po, eng=nc.gpsimd),
        (0, 6): lambda po: stage_c_group(0, 1, 1, po),
        (1, 3): lambda po: stage_c_group(1, 0, 0, po, eng=nc.gpsimd),
        (1, 5): lambda po: stage_c_group(1, 1, 0, po, eng=nc.scalar),
    }

    # ---------- attention ----------
    eb3 = eb_sb[:].rearrange("p (h w) -> p h w", w=GW)
    pending_pv = None

    def emit_pv(args):
        b_, hl_, ng_, kt_, se_, ot_, den_, first, last = args
        for j in range(ng_):
            nc.tensor.matmul(
                ot_[32 * j : 32 * (j + 1), :],
                lhsT=v_sb[b_][kt_][:, 33 * (hl_ + j) : 33 * (hl_ + j) + 32],
                rhs=se_[:, 512 * j : 512 * (j + 1)],
                start=first,
                stop=last,
                tile_position=(0, 32 * j),
                skip_group_check=True,
            )
        for j in range(ng_):
            nc.tensor.matmul(
                den_[32 * j : 32 * (j + 1), :],
                lhsT=ones32_sb[:],
                rhs=se_[:, 512 * j : 512 * (j + 1)],
                start=first,
                stop=last,
                tile_position=(0, 32 * j),
                skip_group_check=True,
            )

    # otn destination mapping: group g rows -> (chunk, chunk-rows, ot-rows)
    OTN_MAP = [
        [(0, 0, 96, 0)],          # G0 -> chunk0 rows 0..96
        [(0, 96, 128, 0), (1, 0, 32, 32), (1, 32, 64, 64)],  # G1 -> chunk0 96..128 + chunk1 0..64
        [(1, 64, 128, 0)],        # G2 -> chunk1 64..128
    ]

    def make_norm(b, g, qi, ot_, den_):
        # Split in two drip-able parts so the DVE chain never sits ahead of
        # the next phase's bias-muls in the queue.
        hl, ng = GRP[g]
        rows = 32 * ng
        state = {}

        def part1():
            rden = nrm_pool.tile([128, 512], F32, tag="rden", name="rden_t")
            rscr = nrm_pool.tile([128, 512], F32, tag="rscr", name="rscr_t")
            nc.vector.reciprocal_approx_accurate(
                out=rden[0:rows, :], in_=den_[0:rows, :], scratch=rscr[0:rows, :]
            )
            state["rden"] = rden

        def part2():
            rden = state["rden"]
            for ch, r0, r1, s0 in OTN_MAP[g]:
                nc.vector.tensor_mul(
                    out=otn_sb[b][ch][r0:r1, 512 * qi : 512 * (qi + 1)],
                    in0=ot_[s0 : s0 + (r1 - r0), :],
                    in1=rden[s0 : s0 + (r1 - r0), :],
                )

        return [part1, part2]

    # Deferred-work queue drained <=2 items per strip-iteration, so the
    # trailing PV packs and norm ops of a phase interleave with the NEXT
    # phase's QK^T matmuls instead of piling up ahead of them at the
    # boundary.  Items: ("pv", args, eligible_iter) / ("fn", closure, iter).
    # PV of strip kt is eligible at kt+2 (its bias-mul is then guaranteed
    # complete, so it never head-of-line blocks the PE queue); the kt==0 PV
    # additionally sits behind the previous phase's norm (+3).
    evq = []
    it = 0

    def drain(limit=2):
        # limit counts PV packs only; norm closures (DVE work) drain freely
        n = 0
        while evq and n < limit and evq[0][2] <= it:
            kind, payload, _ = evq.pop(0)
            if kind == "pv":
                emit_pv(payload)
                n += 1
            else:
                payload()

    phases = [(b, g, qi) for b in range(B_LOC) for g in range(3) for qi in range(2)]
    for pi, (b, g, qi) in enumerate(phases):
        hl, ng = GRP[g]
        w = 512 * ng
        last_phase = pi == len(phases) - 1
        ot_ps = ps_ot.tile([128, 512], F32, tag="ot", name="ot_ps")
        den_ps = ps_den.tile([128, 512], F32, tag="den", name="den_ps")
        for kt in range(8):
            stt = ps_st.tile([128, 1536], F32, tag="st", name="st_ps")
            for j in range(ng):
                h = hl + j
                ch, band = h // 4, 32 * (h % 4)
                nc.tensor.matmul(
                    stt[:, 512 * j : 512 * (j + 1)],
                    lhsT=kT_sb[b][ch][band : band + 32, 128 * kt : 128 * (kt + 1)],
                    rhs=qT_sb[b][ch][band : band + 32, 512 * qi : 512 * (qi + 1)],
                    start=True,
                    stop=True,
                    tile_position=(band, 0),
                )
            if g == 2:
                drip = None
                if b == 0 and b0_drips:
                    drip = b0_drips.pop(0)
                elif b == 1:
                    drip = b1_drips.pop((qi, kt), None)
                if drip is not None:
                    drip(stt[:, 1024:1536])
            elif pi < 2:
                drip = p01_drips.pop((pi, kt), None)
                if drip is not None:
                    # alternate the two accumulator banks; PVs of this phase
                    # start only at cell 6, after which the banks are theirs
                    drip(ot_ps[:] if kt % 2 == 0 else den_ps[:])
            se = se_pool.tile([128, 1536], BF16, tag="se", name="se_t")
            nc.scalar.activation(out=se[:, 0:w], in_=stt[:, 0:w], func=AF.Exp)
            off = 896 - 128 * kt + 512 * qi
            se3 = se[:, 0:w].rearrange("p (j q) -> p j q", q=512)
            nc.vector.tensor_mul(
                out=se3, in0=se3, in1=eb3[:, hl : hl + ng, off : off + 512]
            )
            drain(3 if pi >= len(phases) - 2 else 2)
            if pi < 2:
                elig = it - kt + max(kt + 2, 6)
            elif last_phase and kt >= 5:
                elig = it + 1
            else:
                elig = it + (3 if kt == 0 else 2)
            evq.append(("pv", (b, hl, ng, kt, se, ot_ps, den_ps, kt == 0, kt == 7),
                        elig))
            it += 1
        if last_phase:
            while evq:
                kind, payload, _ = evq.pop(0)
                emit_pv(payload) if kind == "pv" else payload()
            for p in make_norm(b, g, qi, ot_ps, den_ps):
                p()
        else:
            for p in make_norm(b, g, qi, ot_ps, den_ps):
                evq.append(("fn", p, it))

    # ---------- tail: b1's second-half outputs ----------
    stage_c_group(1, 0, 1, ps_ot.tile([128, 512], F32, tag="ot", name="ot_ps"))
    stage_c_group(1, 1, 1, ps_den.tile([128, 512], F32, tag="den", name="den_ps"),
                  eng=nc.scalar)


def build():
    nc = bacc.Bacc("TRN2", target_bir_lowering=False, debug=False, num_devices=N_CORES)
    io = {
        "x": nc.dram_tensor("x", [B_LOC, C, N], BF16, kind="ExternalInput").ap(),
        "wqT": nc.dram_tensor("wqT", [C, C], BF16, kind="ExternalInput").ap(),
        "wkT": nc.dram_tensor("wkT", [C, C], BF16, kind="ExternalInput").ap(),
        "wvT": nc.dram_tensor("wvT", [C, C], BF16, kind="ExternalInput").ap(),
        "woT": nc.dram_tensor("woT", [C, C], BF16, kind="ExternalInput").ap(),
        "bo": nc.dram_tensor("bo", [1, C], BF16, kind="ExternalInput").ap(),
        "eb": nc.dram_tensor("eb", [HEADS, 128, GW], BF16, kind="ExternalInput").ap(),
        "out": nc.dram_tensor("out", [B_LOC, C, N], F32, kind="ExternalOutput").ap(),
    }
    with tile.TileContext(nc) as tc:
        _emit(tc, io)
    nc.compile()
    return nc


_CACHE: dict = {}


def _get_nc():
    if "nc" not in _CACHE:
        _CACHE["nc"] = build()
    return _CACHE["nc"]


def make_in_maps(x, Wq, Wk, Wv, Wo, bo, rel_bias, rel_idx=None):
    """Host-side sharding/layout prep. Returns per-core input maps."""
    import ml_dtypes

    bf16 = ml_dtypes.bfloat16
    x = np.asarray(x, np.float32)
    b, c, H, W = x.shape
    assert (b, c, H * W) == (B_LOC * N_CORES, C, N)
    xr = np.ascontiguousarray(x.reshape(b, c, N).astype(bf16))
    wqT = np.ascontiguousarray(np.asarray(Wq, np.float32).T.astype(bf16))
    wkT = np.ascontiguousarray((np.asarray(Wk, np.float32) * SCALE).T.astype(bf16))
    wvT = np.ascontiguousarray(np.asarray(Wv, np.float32).T.astype(bf16))
    woT = np.ascontiguousarray(np.asarray(Wo, np.float32).T.astype(bf16))
    bo2 = np.ascontiguousarray(np.asarray(bo, np.float32).reshape(1, C).astype(bf16))
    rb = np.asarray(rel_bias, np.float32)
    idx = G0 + np.arange(128)[:, None] - np.arange(GW)[None, :]
    ebmat = np.ascontiguousarray(np.exp(rb[:, idx]).astype(bf16))  # [8, 128, GW]
    shared = dict(wqT=wqT, wkT=wkT, wvT=wvT, woT=woT, bo=bo2, eb=ebmat)
    return [
        dict(x=np.ascontiguousarray(xr[B_LOC * i : B_LOC * (i + 1)]), **shared)
        for i in range(N_CORES)
    ]


def _install_ntff_hook_shim():
    """bass_utils fetches the axon NTFF hook via antenv.axon_hooks, which this
    image's antenv lacks; synthesize it from trn_agent_boot's ctypes hook."""
    import sys
    import types

    try:
        from antenv.axon_hooks import get_axon_ntff_profile_hook  # noqa: F401

        return
    except ImportError:
        pass
    hook = None
    try:
        from trn_agent_boot.trn_boot import _ntff_profile_via_ctypes

        hook = _ntff_profile_via_ctypes("/opt/axon/libaxon_pjrt.so")
    except Exception:
        pass
    mod = types.ModuleType("antenv.axon_hooks")
    state = {"hook": hook}
    mod.get_axon_ntff_profile_hook = lambda: state["hook"]
    mod.set_axon_ntff_profile_hook = lambda h: state.__setitem__("hook", h)
    sys.modules["antenv.axon_hooks"] = mod


def run(inputs: dict, trace: bool = False):
    """Run on the 8 cores; returns (full_output, BassKernelResults)."""
    if trace:
        _install_ntff_hook_shim()
    in_maps = make_in_maps(**inputs)
    nc = _get_nc()
    res = bass_utils.run_bass_kernel_spmd(
        nc, in_maps, core_ids=list(range(N_CORES)), trace=trace
    )
    outs = np.stack([res.results[i]["out"] for i in range(N_CORES)])
    out = outs.reshape(B_LOC * N_CORES, C, 32, 32)
    return out, res


def kernel(**inputs) -> np.ndarray:
    out, _ = run(inputs)
    return out
